# revision 31
# baseline (speedup 1.0000x reference)
"""MoE layer (E=8, top-2, H=2048, I=5120, 2 shared experts) on 8 Trainium2 cores.

Sharding strategy (expert-parallel, per the hint):
  - Router runs on host (it is the *sharding function*: expert-parallel
    dispatch of full inputs requires the routing decision at shard time).
    It is computed with CPU jax using exactly the reference ops so the
    top-k selection (heavily tie-dominated: ~36% of sigmoid scores
    saturate to exactly 1.0) matches the oracle bit-for-bit.
  - The 2*T (token, expert) pairs are split into 16 single-expert pieces
    of <= L tokens; each of the 8 cores gets two pieces (segments A/B),
    each with its own expert-weight inputs.  This load-balances the very
    skewed expert distribution under an SPMD (single-program) kernel.
  - Shared experts are tensor-parallel: the flattened [NSH*I] = 10240
    intermediate dim is sliced 1280 per core; partial outputs summed on
    host during unshard.
  - On-device math: fp16 inputs/weights, fp32 PSUM accumulation
    (measured ~5e-4 max rel err vs the fp32 oracle on host).
  - All device inputs are host-repacked into partition-major tile
    layouts so every DMA is a large per-partition-contiguous transfer.
"""

import os
import subprocess
import sys
import tempfile

import numpy as np

# Problem constants (hardcoded per contract - kernel.py is self-contained).
B, S, H, I, E, TOPK, NSH = 2, 1024, 2048, 5120, 8, 2, 2
T = B * S                      # 2048 tokens
ISH = NSH * I                  # 10240 flattened shared intermediate dim
NCORES = 8
ISHC = ISH // NCORES           # 1280 shared intermediate per core
KT = H // 128                  # 16 contraction tiles over H
NITG = I // 512                # 10 groups of 4 i-tiles
NIT = I // 128                 # 40 i-tiles
NHT = H // 128                 # 16 output h-tiles
NHTG = H // 256                # 8 pairs of h-tiles
SIT = ISHC // 128              # 10 shared i-tiles per core
NCH = T // 512                 # 4 chunks of 512 tokens (shared phase)

F16NP = np.float16

_ROUTE_SRC = """
import jax
jax.config.update("jax_platforms", "cpu")
import jax.numpy as jnp, numpy as np, sys
d = np.load(sys.argv[1])
xt = jnp.asarray(d["xt"]); wr = jnp.asarray(d["wr"])
logits = jnp.einsum("th,eh->te", xt, wr)
scores = jax.nn.sigmoid(logits)
topw, topi = jax.lax.top_k(scores, 2)
topw = topw / jnp.sum(topw, axis=-1, keepdims=True)
np.savez(sys.argv[2], topi=np.asarray(topi), topw=np.asarray(topw))
"""


def _route_cpu(xt, wr):
    """Top-2 routing with CPU jax in a subprocess (the main process's jax
    backend is axon; the oracle's routing is CPU-jax and tie-break
    sensitive, so it must be reproduced bit-exactly on the same backend)."""
    with tempfile.TemporaryDirectory() as td:
        inp = os.path.join(td, "in.npz")
        outp = os.path.join(td, "out.npz")
        src = os.path.join(td, "route.py")
        np.savez(inp, xt=xt, wr=wr)
        with open(src, "w") as f:
            f.write(_ROUTE_SRC)
        env = dict(os.environ)
        env.pop("JAX_PLATFORMS", None)
        subprocess.run([sys.executable, src, inp, outp], check=True, env=env)
        d = np.load(outp)
        return d["topi"], d["topw"]


def _try_assign(counts, LA, LB):
    """Assign each expert (a_e, b_e) slot counts with a_e*LA + b_e*LB >=
    N_e, sum(a) <= 8, sum(b) <= 8.  DFS over experts, largest first."""
    order = sorted(range(E), key=lambda e: -counts[e])

    def rec(idx, remA, remB, acc):
        if idx == len(order):
            return acc
        e = order[idx]
        n = int(counts[e])
        for a in range(min(remA, -(-n // LA)) + 1):
            rem = n - a * LA
            b = max(0, -(-rem // LB))
            if b > remB:
                continue
            r = rec(idx + 1, remA - a, remB - b, acc + [(e, a, b)])
            if r is not None:
                return r
        return None

    return rec(0, NCORES, NCORES, [])


def _plan_segments(counts):
    """Smallest per-core capacity (LA, LB) with a feasible slot assignment.
    LA/LB are compile-time segment lengths (<=512, multiples of 64)."""
    for tot in range(128, 2 * 512 + 1, 64):
        for LB in range(min(tot // 2 // 64 * 64, 512), 63, -64):
            LA = tot - LB
            if LA > 512:
                continue
            asg = _try_assign(counts, LA, LB)
            if asg is not None:
                return LA, LB, asg
    raise ValueError(f"cannot pack counts {counts}")


# ------------- host-side packing into partition-major DMA layouts -----------

def _pack_gu(w):            # w [I, H] f32 -> [NITG, 128, KT, 512] f16
    wT = np.ascontiguousarray(w.T).astype(F16NP)            # [H, I]
    return np.ascontiguousarray(
        wT.reshape(KT, 128, NITG, 512).transpose(2, 1, 0, 3))


def _pack_d(w):             # w [H, I] f32 -> [NHTG, 128, NIT, 256] f16
    wT = np.ascontiguousarray(w.T).astype(F16NP)            # [I, H]
    return np.ascontiguousarray(
        wT.reshape(NIT, 128, NHTG, 256).transpose(2, 1, 0, 3))


def _pack_sh_gu(sgc):       # sgc [ISHC, H] f32 -> [SIT, 128, KT, 128] f16
    sT = np.ascontiguousarray(sgc.T).astype(F16NP)          # [H, ISHC]
    return np.ascontiguousarray(
        sT.reshape(KT, 128, SIT, 128).transpose(2, 1, 0, 3))


def _pack_sh_d(sdc):        # sdc [H, ISHC] f32 -> [NHTG, 128, SIT, 256] f16
    sT = np.ascontiguousarray(sdc.T).astype(F16NP)          # [ISHC, H]
    return np.ascontiguousarray(
        sT.reshape(SIT, 128, NHTG, 256).transpose(2, 1, 0, 3))


def _pack_x_slot(xt16T, tokens, L):   # -> [128, KT, L] f16
    xs = np.zeros((H, L), F16NP)
    if len(tokens):
        xs[:, : len(tokens)] = xt16T[:, tokens]
    return np.ascontiguousarray(xs.reshape(KT, 128, L).transpose(1, 0, 2))


# --------------------------- device program ---------------------------------

def _build_nc(LA, LB):
    import concourse.mybir as mybir
    import concourse.tile as tile
    from concourse import bacc

    F16 = mybir.dt.float16
    F32 = mybir.dt.float32
    SILU = mybir.ActivationFunctionType.Silu

    nc = bacc.Bacc("TRN2", debug=False, num_devices=NCORES)

    def din(name, shape):
        return nc.dram_tensor(name, shape, F16, kind="ExternalInput").ap()

    xa = din("xa", [128, KT, LA])
    xb = din("xb", [128, KT, LB])
    wg = [din(f"wg{s}", [NITG, 128, KT, 512]) for s in "ab"]
    wu = [din(f"wu{s}", [NITG, 128, KT, 512]) for s in "ab"]
    wd = [din(f"wd{s}", [NHTG, 128, NIT, 256]) for s in "ab"]
    xall = din("xall", [NCH, 128, KT, 512])
    sg = din("sg", [SIT, 128, KT, 128])
    su = din("su", [SIT, 128, KT, 128])
    sd = din("sd", [NHTG, 128, SIT, 256])

    yt = nc.dram_tensor("yt", [H, LA + LB], F32, kind="ExternalOutput").ap()
    ysh = nc.dram_tensor("ysh", [H, T], F32, kind="ExternalOutput").ap()

    with tile.TileContext(nc) as tc, \
            tc.tile_pool(name="bridge", bufs=1) as bridge:
        # bridge tiles span the routed->shared pool-scope boundary so the
        # shared phase's first iteration has its inputs prefetched and its
        # compute emitted inside the routed scope (no transition stall)
        bxc0 = bridge.tile([128, KT, 512], F16, name="bxc0", tag="bxc")
        bsg = bridge.tile([128, KT, 128], F16, name="bsg", tag="bsg")
        bsu = bridge.tile([128, KT, 128], F16, name="bsu", tag="bsu")
        bh0 = bridge.tile([128, 512], F16, name="bh0", tag="bh0")
        # ---------------- routed experts: two segments ----------------
        with (
            tc.tile_pool(name="psum", bufs=2, space="PSUM") as psum,
            tc.tile_pool(name="xslot", bufs=2) as xpool,
            tc.tile_pool(name="wgu", bufs=3) as wpool,
            tc.tile_pool(name="wdp", bufs=2) as wdpool,
            tc.tile_pool(name="hp", bufs=2) as hpool,
            tc.tile_pool(name="tmp", bufs=3) as tmppool,
            tc.tile_pool(name="osb", bufs=2) as opool,
        ):
            for seg, (xs_d, L, off) in enumerate(
                    [(xa, LA, 0), (xb, LB, LA)]):
                xs = xpool.tile([128, KT, L], F16, name="xs", tag="xs")
                if seg > 0:
                    nc.sync.dma_start(out=xs, in_=xs_d)
                h_all = hpool.tile([128, NIT, L], F16, name="h_all", tag="h")
                # stage 1: h = silu(x@wgT) * (x@wuT), laid out [I, L].
                # weights streamed in half-ktile tiles for finer prefetch
                for itg in range(NITG):
                    wgh = [wpool.tile([128, KT // 2, 512], F16,
                                      name=f"wgt{hh}", tag="wg")
                           for hh in range(2)]
                    wuh = [wpool.tile([128, KT // 2, 512], F16,
                                      name=f"wut{hh}", tag="wu")
                           for hh in range(2)]
                    if seg == 0 and itg == 0:
                        # interleave first loads in 4-ktile chunks so the
                        # first matmuls' inputs arrive as early as possible
                        for q in range(4):
                            hh, lq = q // 2, q % 2
                            dsl = slice(q * 4, (q + 1) * 4)
                            lsl = slice(lq * 4, (lq + 1) * 4)
                            nc.sync.dma_start(out=xs[:, dsl, :],
                                              in_=xs_d[:, dsl, :])
                            nc.sync.dma_start(out=wgh[hh][:, lsl, :],
                                              in_=wg[seg][itg, :, dsl, :])
                            nc.sync.dma_start(out=wuh[hh][:, lsl, :],
                                              in_=wu[seg][itg, :, dsl, :])
                        # prefetch the shared-phase bridge inputs early
                        nc.sync.dma_start(out=bxc0, in_=xall[0])
                        nc.sync.dma_start(out=bsg, in_=sg[0])
                        nc.sync.dma_start(out=bsu, in_=su[0])
                    else:
                        for hh in range(2):
                            hsl = slice(hh * 8, (hh + 1) * 8)
                            nc.sync.dma_start(out=wgh[hh],
                                              in_=wg[seg][itg, :, hsl, :])
                            nc.sync.dma_start(out=wuh[hh],
                                              in_=wu[seg][itg, :, hsl, :])
                    for it4 in range(4):
                        it = itg * 4 + it4
                        pg = psum.tile([128, L], F32, name="pg", tag="pg",
                                       space="PSUM")
                        pu = psum.tile([128, L], F32, name="pu", tag="pu",
                                       space="PSUM")
                        csl = slice(it4 * 128, (it4 + 1) * 128)
                        for kt in range(KT):
                            nc.tensor.matmul(pg,
                                             lhsT=wgh[kt // 8][:, kt % 8, csl],
                                             rhs=xs[:, kt, :],
                                             start=(kt == 0),
                                             stop=(kt == KT - 1))
                        for kt in range(KT):
                            nc.tensor.matmul(pu,
                                             lhsT=wuh[kt // 8][:, kt % 8, csl],
                                             rhs=xs[:, kt, :],
                                             start=(kt == 0),
                                             stop=(kt == KT - 1))
                        gsb = tmppool.tile([128, L], F16, name="gsb",
                                           tag="gsb")
                        nc.scalar.activation(out=gsb, in_=pg, func=SILU)
                        nc.vector.tensor_mul(out=h_all[:, it, :], in0=gsb,
                                             in1=pu)
                # stage 2: y = h @ wdT, output [H, L].  wd streamed in
                # half-tiles (finer prefetch rotation hides DMA latency)
                NH2 = NIT // 2
                for htg in range(NHTG):
                    wdt0 = wdpool.tile([128, NH2, 256], F16, name="wdt0",
                                       tag="wd", bufs=4)
                    nc.sync.dma_start(out=wdt0, in_=wd[seg][htg, :, :NH2, :])
                    wdt1 = wdpool.tile([128, NH2, 256], F16, name="wdt1",
                                       tag="wd", bufs=4)
                    nc.sync.dma_start(out=wdt1, in_=wd[seg][htg, :, NH2:, :])
                    py0 = psum.tile([128, L], F32, name="py0", tag="py",
                                    bufs=4, space="PSUM")
                    py1 = psum.tile([128, L], F32, name="py1", tag="py",
                                    bufs=4, space="PSUM")
                    for it in range(NIT):
                        wdt = wdt0 if it < NH2 else wdt1
                        i2 = it % NH2
                        nc.tensor.matmul(py0, lhsT=wdt[:, i2, 0:128],
                                         rhs=h_all[:, it, :],
                                         start=(it == 0), stop=(it == NIT - 1))
                        nc.tensor.matmul(py1, lhsT=wdt[:, i2, 128:256],
                                         rhs=h_all[:, it, :],
                                         start=(it == 0), stop=(it == NIT - 1))
                    ysb = opool.tile([128, 2, L], F32, name="ysb", tag="ysb")
                    nc.vector.tensor_copy(out=ysb[:, 0, :], in_=py0)
                    nc.vector.tensor_copy(out=ysb[:, 1, :], in_=py1)
                    nc.sync.dma_start(
                        out=yt[htg * 256:(htg + 1) * 256,
                               off:off + L].rearrange(
                            "(s p) l -> p s l", p=128),
                        in_=ysb)

            # shared-phase prologue (it=0, chunk=0) emitted in this scope
            # so it overlaps the pool-scope transition below
            pg = psum.tile([128, 512], F32, name="ppg", tag="pg",
                           space="PSUM")
            pu = psum.tile([128, 512], F32, name="ppu", tag="pu",
                           space="PSUM")
            for kt in range(KT):
                nc.tensor.matmul(pg, lhsT=bsg[:, kt, :], rhs=bxc0[:, kt, :],
                                 start=(kt == 0), stop=(kt == KT - 1))
            for kt in range(KT):
                nc.tensor.matmul(pu, lhsT=bsu[:, kt, :], rhs=bxc0[:, kt, :],
                                 start=(kt == 0), stop=(kt == KT - 1))
            gsb = tmppool.tile([128, 512], F16, name="pgsb", tag="gsb")
            nc.scalar.activation(out=gsb, in_=pg, func=SILU)
            nc.vector.tensor_mul(out=bh0, in0=gsb, in1=pu)

        # ---------------- shared experts (TP slice) ----------------
        with (
            tc.tile_pool(name="psum2", bufs=2, space="PSUM") as psum,
            tc.tile_pool(name="xallp", bufs=3) as xapool,
            tc.tile_pool(name="shw", bufs=3) as shwpool,
            tc.tile_pool(name="hshp", bufs=1) as hshpool,
            tc.tile_pool(name="sdp", bufs=2) as sdpool,
            tc.tile_pool(name="tmp2", bufs=3) as tmppool,
            tc.tile_pool(name="osb2", bufs=4) as opool,
        ):
            xc = [bxc0] + [
                xapool.tile([128, KT, 512], F16, name=f"xct{c}", tag="xc")
                for c in range(1, NCH)]
            hsh = hshpool.tile([128, SIT, NCH, 512], F16, name="hsh",
                               tag="hsh")
            # B1: shared swiglu intermediate for all tokens
            # (it=0, c=0 was computed in the routed scope via bridge tiles)
            for it in range(SIT):
                if it == 0:
                    sgt, sut = bsg, bsu
                    for c in range(1, NCH):
                        nc.sync.dma_start(out=xc[c], in_=xall[c])
                    crange = range(1, NCH)
                else:
                    sgt = shwpool.tile([128, KT, 128], F16, name="sgt",
                                       tag="sg")
                    nc.sync.dma_start(out=sgt, in_=sg[it])
                    sut = shwpool.tile([128, KT, 128], F16, name="sut",
                                       tag="su")
                    nc.sync.dma_start(out=sut, in_=su[it])
                    crange = range(NCH)
                for c in crange:
                    pg = psum.tile([128, 512], F32, name="pg2", tag="pg",
                                   space="PSUM")
                    pu = psum.tile([128, 512], F32, name="pu2", tag="pu",
                                   space="PSUM")
                    for kt in range(KT):
                        nc.tensor.matmul(pg, lhsT=sgt[:, kt, :],
                                         rhs=xc[c][:, kt, :],
                                         start=(kt == 0), stop=(kt == KT - 1))
                    for kt in range(KT):
                        nc.tensor.matmul(pu, lhsT=sut[:, kt, :],
                                         rhs=xc[c][:, kt, :],
                                         start=(kt == 0), stop=(kt == KT - 1))
                    gsb = tmppool.tile([128, 512], F16, name="gsb2",
                                       tag="gsb")
                    nc.scalar.activation(out=gsb, in_=pg, func=SILU)
                    nc.vector.tensor_mul(out=hsh[:, it, c, :], in0=gsb,
                                         in1=pu)
            # B2: down-projection of the shared slice
            for htg in range(NHTG):
                sdt = sdpool.tile([128, SIT, 256], F16, name="sdt", tag="sd")
                nc.sync.dma_start(out=sdt, in_=sd[htg])
                for c in range(NCH):
                    py0 = psum.tile([128, 512], F32, name="spy0", tag="py",
                                    bufs=4, space="PSUM")
                    py1 = psum.tile([128, 512], F32, name="spy1", tag="py",
                                    bufs=4, space="PSUM")
                    for it in range(SIT):
                        rhs = bh0 if (it == 0 and c == 0) \
                            else hsh[:, it, c, :]
                        nc.tensor.matmul(py0, lhsT=sdt[:, it, 0:128],
                                         rhs=rhs,
                                         start=(it == 0), stop=(it == SIT - 1))
                        nc.tensor.matmul(py1, lhsT=sdt[:, it, 128:256],
                                         rhs=rhs,
                                         start=(it == 0), stop=(it == SIT - 1))
                    ysb = opool.tile([128, 2, 512], F32, name="ysb2",
                                     tag="ysb")
                    nc.vector.tensor_copy(out=ysb[:, 0, :], in_=py0)
                    nc.vector.tensor_copy(out=ysb[:, 1, :], in_=py1)
                    nc.sync.dma_start(
                        out=ysh[htg * 256:(htg + 1) * 256,
                                c * 512:(c + 1) * 512].rearrange(
                            "(s p) l -> p s l", p=128),
                        in_=ysb)
    nc.compile()
    return nc


def _ensure_ntff_hook():
    """This image's `antenv` package is a stub without `axon_hooks`, which
    makes run_bass_kernel_spmd(trace=True) crash on import even though the
    axon .so supports NTFF capture.  Provide the module and register the
    ctypes-based hook from trn_agent_boot (no-op if anything is missing)."""
    try:
        import antenv.axon_hooks  # noqa: F401
        return
    except ImportError:
        pass
    try:
        import types

        import antenv
        from trn_agent_boot.trn_boot import _ntff_profile_via_ctypes

        mod = types.ModuleType("antenv.axon_hooks")
        mod._hook = None

        def set_axon_ntff_profile_hook(h):
            mod._hook = h

        def get_axon_ntff_profile_hook():
            return mod._hook

        mod.set_axon_ntff_profile_hook = set_axon_ntff_profile_hook
        mod.get_axon_ntff_profile_hook = get_axon_ntff_profile_hook
        sys.modules["antenv.axon_hooks"] = mod
        antenv.axon_hooks = mod
        mod._hook = _ntff_profile_via_ctypes("/opt/axon/libaxon_pjrt.so")
    except Exception:
        pass


LAST_EXEC_NS = None
LAST_RESULTS = None


def kernel(x, w_router, wg, wu, wd, sg, su, sd):
    global LAST_EXEC_NS, LAST_RESULTS
    from concourse import bass_utils

    x = np.asarray(x, dtype=np.float32)
    xt = np.ascontiguousarray(x.reshape(T, H))

    # 1) routing on host (CPU jax, bit-exact with the oracle)
    topi, topw = _route_cpu(xt, np.asarray(w_router, dtype=np.float32))
    counts = np.bincount(topi.reshape(-1), minlength=E)
    LA, LB, asg = _plan_segments(counts)

    # 2) build 8 A-pieces (<= LA tokens) and 8 B-pieces (<= LB tokens),
    #    each single-expert, per the slot assignment
    empty = (0, np.zeros(0, np.int64), np.zeros(0, np.float32))
    piecesA, piecesB = [], []
    for e, a, b in asg:
        ts = np.where((topi == e).any(axis=1))[0]
        w = np.where(topi[ts, 0] == e, topw[ts, 0], topw[ts, 1]).astype(
            np.float32)
        pos = 0
        for _ in range(a):
            n = min(LA, len(ts) - pos)
            if n > 0:
                piecesA.append((e, ts[pos:pos + n], w[pos:pos + n]))
                pos += n
            else:
                piecesA.append(empty)
        for _ in range(b):
            n = min(LB, len(ts) - pos)
            if n > 0:
                piecesB.append((e, ts[pos:pos + n], w[pos:pos + n]))
                pos += n
            else:
                piecesB.append(empty)
        assert pos == len(ts), (e, pos, len(ts))
    assert len(piecesA) <= NCORES and len(piecesB) <= NCORES
    piecesA += [empty] * (NCORES - len(piecesA))
    piecesB += [empty] * (NCORES - len(piecesB))

    # 3) per-core device inputs
    xt16T = np.ascontiguousarray(xt.T).astype(F16NP)        # [H, T]
    wcache = {}

    def expert_pack(e):
        if e not in wcache:
            wcache[e] = (_pack_gu(np.asarray(wg[e])),
                         _pack_gu(np.asarray(wu[e])),
                         _pack_d(np.asarray(wd[e])))
        return wcache[e]

    xall_dev = np.ascontiguousarray(
        xt16T.reshape(KT, 128, NCH, 512).transpose(2, 1, 0, 3))

    SG = np.asarray(sg, dtype=np.float32).reshape(ISH, H)
    SU = np.asarray(su, dtype=np.float32).reshape(ISH, H)
    SD = np.ascontiguousarray(
        np.asarray(sd, dtype=np.float32).transpose(1, 0, 2).reshape(H, ISH))

    in_maps = []
    for k in range(NCORES):
        m = {"xall": xall_dev,
             "sg": _pack_sh_gu(SG[k * ISHC:(k + 1) * ISHC]),
             "su": _pack_sh_gu(SU[k * ISHC:(k + 1) * ISHC]),
             "sd": _pack_sh_d(SD[:, k * ISHC:(k + 1) * ISHC])}
        for sn, L, pieces in (("a", LA, piecesA), ("b", LB, piecesB)):
            e, ts, _ = pieces[k]
            g, u, dn = expert_pack(e)
            m[f"x{sn}"] = _pack_x_slot(xt16T, ts, L)
            m[f"wg{sn}"] = g
            m[f"wu{sn}"] = u
            m[f"wd{sn}"] = dn
        in_maps.append(m)

    # 4) compile + run on the 8 cores
    nc = _build_nc(LA, LB)
    trace = os.environ.get("MOE_TRACE", "0") == "1"
    _ensure_ntff_hook()
    res = bass_utils.run_bass_kernel_spmd(
        nc, in_maps, core_ids=list(range(NCORES)), trace=trace)
    LAST_EXEC_NS = res.exec_time_ns
    LAST_RESULTS = res

    # 5) unshard: weighted scatter-add of routed pieces + sum of shared parts
    outT = np.zeros((H, T), np.float32)
    for k in range(NCORES):
        outT += res.results[k]["ysh"]
    for pieces, off in ((piecesA, 0), (piecesB, LA)):
        for k, (e, ts, w) in enumerate(pieces):
            if len(ts) == 0:
                continue
            yseg = res.results[k]["yt"][:, off:off + len(ts)]   # [H, n]
            outT[:, ts] += yseg * w[None, :]

    return np.ascontiguousarray(outT.T).reshape(B, S, H).astype(np.float32)


# revision 33
# speedup vs baseline: 1.0187x; 1.0187x over previous
"""MoE layer (E=8, top-2, H=2048, I=5120, 2 shared experts) on 8 Trainium2 cores.

Sharding strategy (expert-parallel, per the hint):
  - Router runs on host (it is the *sharding function*: expert-parallel
    dispatch of full inputs requires the routing decision at shard time).
    It is computed with CPU jax using exactly the reference ops so the
    top-k selection (heavily tie-dominated: ~36% of sigmoid scores
    saturate to exactly 1.0) matches the oracle bit-for-bit.
  - The 2*T (token, expert) pairs are split into 16 single-expert pieces
    of <= L tokens; each of the 8 cores gets two pieces (segments A/B),
    each with its own expert-weight inputs.  This load-balances the very
    skewed expert distribution under an SPMD (single-program) kernel.
  - Shared experts are tensor-parallel: the flattened [NSH*I] = 10240
    intermediate dim is sliced 1280 per core; partial outputs summed on
    host during unshard.
  - On-device math: fp16 inputs/weights, fp32 PSUM accumulation
    (measured ~5e-4 max rel err vs the fp32 oracle on host).
  - All device inputs are host-repacked into partition-major tile
    layouts so every DMA is a large per-partition-contiguous transfer.
"""

import os
import subprocess
import sys
import tempfile

import numpy as np

# Problem constants (hardcoded per contract - kernel.py is self-contained).
B, S, H, I, E, TOPK, NSH = 2, 1024, 2048, 5120, 8, 2, 2
T = B * S                      # 2048 tokens
ISH = NSH * I                  # 10240 flattened shared intermediate dim
NCORES = 8
ISHC = ISH // NCORES           # 1280 shared intermediate per core
KT = H // 128                  # 16 contraction tiles over H
NITG = I // 512                # 10 groups of 4 i-tiles
NIT = I // 128                 # 40 i-tiles
NHT = H // 128                 # 16 output h-tiles
NHTG = H // 256                # 8 pairs of h-tiles
SIT = ISHC // 128              # 10 shared i-tiles per core
NCH = T // 512                 # 4 chunks of 512 tokens (shared phase)

F16NP = np.float16

_ROUTE_SRC = """
import jax
jax.config.update("jax_platforms", "cpu")
import jax.numpy as jnp, numpy as np, sys
d = np.load(sys.argv[1])
xt = jnp.asarray(d["xt"]); wr = jnp.asarray(d["wr"])
logits = jnp.einsum("th,eh->te", xt, wr)
scores = jax.nn.sigmoid(logits)
topw, topi = jax.lax.top_k(scores, 2)
topw = topw / jnp.sum(topw, axis=-1, keepdims=True)
np.savez(sys.argv[2], topi=np.asarray(topi), topw=np.asarray(topw))
"""


def _route_cpu(xt, wr):
    """Top-2 routing with CPU jax in a subprocess (the main process's jax
    backend is axon; the oracle's routing is CPU-jax and tie-break
    sensitive, so it must be reproduced bit-exactly on the same backend)."""
    with tempfile.TemporaryDirectory() as td:
        inp = os.path.join(td, "in.npz")
        outp = os.path.join(td, "out.npz")
        src = os.path.join(td, "route.py")
        np.savez(inp, xt=xt, wr=wr)
        with open(src, "w") as f:
            f.write(_ROUTE_SRC)
        env = dict(os.environ)
        env.pop("JAX_PLATFORMS", None)
        subprocess.run([sys.executable, src, inp, outp], check=True, env=env)
        d = np.load(outp)
        return d["topi"], d["topw"]


def _try_assign(counts, LA, LB):
    """Assign each expert (a_e, b_e) slot counts with a_e*LA + b_e*LB >=
    N_e, sum(a) <= 8, sum(b) <= 8.  DFS over experts, largest first."""
    order = sorted(range(E), key=lambda e: -counts[e])

    def rec(idx, remA, remB, acc):
        if idx == len(order):
            return acc
        e = order[idx]
        n = int(counts[e])
        for a in range(min(remA, -(-n // LA)) + 1):
            rem = n - a * LA
            b = max(0, -(-rem // LB))
            if b > remB:
                continue
            r = rec(idx + 1, remA - a, remB - b, acc + [(e, a, b)])
            if r is not None:
                return r
        return None

    return rec(0, NCORES, NCORES, [])


def _plan_segments(counts):
    """Smallest per-core capacity (LA, LB) with a feasible slot assignment.
    LA/LB are compile-time segment lengths (<=512, multiples of 64)."""
    for tot in range(128, 2 * 512 + 1, 64):
        for LB in range(min(tot // 2 // 64 * 64, 512), 63, -64):
            LA = tot - LB
            if LA > 512:
                continue
            asg = _try_assign(counts, LA, LB)
            if asg is not None:
                return LA, LB, asg
    raise ValueError(f"cannot pack counts {counts}")


# ------------- host-side packing into partition-major DMA layouts -----------

def _pack_gu(w):            # w [I, H] f32 -> [NITG, 128, KT, 512] f16
    wT = np.ascontiguousarray(w.T).astype(F16NP)            # [H, I]
    return np.ascontiguousarray(
        wT.reshape(KT, 128, NITG, 512).transpose(2, 1, 0, 3))


def _pack_d(w):             # w [H, I] f32 -> [NHTG, 128, NIT, 256] f16
    wT = np.ascontiguousarray(w.T).astype(F16NP)            # [I, H]
    return np.ascontiguousarray(
        wT.reshape(NIT, 128, NHTG, 256).transpose(2, 1, 0, 3))


def _pack_sh_gu(sgc):       # sgc [ISHC, H] f32 -> [SIT, 128, KT, 128] f16
    sT = np.ascontiguousarray(sgc.T).astype(F16NP)          # [H, ISHC]
    return np.ascontiguousarray(
        sT.reshape(KT, 128, SIT, 128).transpose(2, 1, 0, 3))


def _pack_sh_d(sdc):        # sdc [H, ISHC] f32 -> [NHTG, 128, SIT, 256] f16
    sT = np.ascontiguousarray(sdc.T).astype(F16NP)          # [ISHC, H]
    return np.ascontiguousarray(
        sT.reshape(SIT, 128, NHTG, 256).transpose(2, 1, 0, 3))


def _pack_x_slot(xt16T, tokens, L):   # -> [128, KT, L] f16
    xs = np.zeros((H, L), F16NP)
    if len(tokens):
        xs[:, : len(tokens)] = xt16T[:, tokens]
    return np.ascontiguousarray(xs.reshape(KT, 128, L).transpose(1, 0, 2))


# --------------------------- device program ---------------------------------

def _build_nc(LA, LB):
    import concourse.mybir as mybir
    import concourse.tile as tile
    from concourse import bacc

    F16 = mybir.dt.float16
    F32 = mybir.dt.float32
    SILU = mybir.ActivationFunctionType.Silu

    nc = bacc.Bacc("TRN2", debug=False, num_devices=NCORES)

    def din(name, shape):
        return nc.dram_tensor(name, shape, F16, kind="ExternalInput").ap()

    xa = din("xa", [128, KT, LA])
    xb = din("xb", [128, KT, LB])
    wg = [din(f"wg{s}", [NITG, 128, KT, 512]) for s in "ab"]
    wu = [din(f"wu{s}", [NITG, 128, KT, 512]) for s in "ab"]
    wd = [din(f"wd{s}", [NHTG, 128, NIT, 256]) for s in "ab"]
    xall = din("xall", [NCH, 128, KT, 512])
    sg = din("sg", [SIT, 128, KT, 128])
    su = din("su", [SIT, 128, KT, 128])
    sd = din("sd", [NHTG, 128, SIT, 256])

    yt = nc.dram_tensor("yt", [H, LA + LB], F32, kind="ExternalOutput").ap()
    ysh = nc.dram_tensor("ysh", [H, T], F32, kind="ExternalOutput").ap()

    with tile.TileContext(nc) as tc, \
            tc.tile_pool(name="bridge", bufs=1) as bridge:
        # bridge tiles span the routed->shared pool-scope boundary so the
        # shared phase's first iteration has its inputs prefetched and its
        # compute emitted inside the routed scope (no transition stall)
        bxc0 = bridge.tile([128, KT, 512], F16, name="bxc0", tag="bxc")
        bsg = bridge.tile([128, KT, 128], F16, name="bsg", tag="bsg")
        bsu = bridge.tile([128, KT, 128], F16, name="bsu", tag="bsu")
        bh0 = bridge.tile([128, 512], F16, name="bh0", tag="bh0")
        # ---------------- routed experts: two segments ----------------
        with (
            tc.tile_pool(name="psum", bufs=2, space="PSUM") as psum,
            tc.tile_pool(name="xslot", bufs=2) as xpool,
            tc.tile_pool(name="wgu", bufs=2) as wpool,
            tc.tile_pool(name="wdp", bufs=2) as wdpool,
            tc.tile_pool(name="hp", bufs=2) as hpool,
            tc.tile_pool(name="tmp", bufs=3) as tmppool,
            tc.tile_pool(name="osb", bufs=2) as opool,
        ):
            for seg, (xs_d, L, off) in enumerate(
                    [(xa, LA, 0), (xb, LB, LA)]):
                xs = xpool.tile([128, KT, L], F16, name="xs", tag="xs")
                if seg > 0:
                    nc.sync.dma_start(out=xs, in_=xs_d)
                h_all = hpool.tile([128, NIT, L], F16, name="h_all", tag="h")
                # stage 1: h = silu(x@wgT) * (x@wuT), laid out [I, L]
                for itg in range(NITG):
                    wgt = wpool.tile([128, KT, 512], F16, name="wgt", tag="wg")
                    wut = wpool.tile([128, KT, 512], F16, name="wut", tag="wu")
                    if seg == 0 and itg == 0:
                        # interleave first loads in 4-ktile chunks so the
                        # first matmuls' inputs arrive as early as possible
                        for q in range(4):
                            ksl = slice(q * 4, (q + 1) * 4)
                            nc.sync.dma_start(out=xs[:, ksl, :],
                                              in_=xs_d[:, ksl, :])
                            nc.sync.dma_start(out=wgt[:, ksl, :],
                                              in_=wg[seg][itg, :, ksl, :])
                            nc.sync.dma_start(out=wut[:, ksl, :],
                                              in_=wu[seg][itg, :, ksl, :])
                        # prefetch the shared-phase bridge inputs early
                        nc.sync.dma_start(out=bxc0, in_=xall[0])
                        nc.sync.dma_start(out=bsg, in_=sg[0])
                        nc.sync.dma_start(out=bsu, in_=su[0])
                    else:
                        nc.sync.dma_start(out=wgt, in_=wg[seg][itg])
                        nc.sync.dma_start(out=wut, in_=wu[seg][itg])
                    for it4 in range(4):
                        it = itg * 4 + it4
                        pg = psum.tile([128, L], F32, name="pg", tag="pg",
                                       space="PSUM")
                        pu = psum.tile([128, L], F32, name="pu", tag="pu",
                                       space="PSUM")
                        csl = slice(it4 * 128, (it4 + 1) * 128)
                        for kt in range(KT):
                            nc.tensor.matmul(pg, lhsT=wgt[:, kt, csl],
                                             rhs=xs[:, kt, :],
                                             start=(kt == 0),
                                             stop=(kt == KT - 1))
                        for kt in range(KT):
                            nc.tensor.matmul(pu, lhsT=wut[:, kt, csl],
                                             rhs=xs[:, kt, :],
                                             start=(kt == 0),
                                             stop=(kt == KT - 1))
                        gsb = tmppool.tile([128, L], F16, name="gsb",
                                           tag="gsb")
                        nc.scalar.activation(out=gsb, in_=pg, func=SILU)
                        nc.vector.tensor_mul(out=h_all[:, it, :], in0=gsb,
                                             in1=pu)
                # stage 2: y = h @ wdT, output [H, L].  wd streamed in
                # half-tiles (finer prefetch rotation hides DMA latency)
                NH2 = NIT // 2
                for htg in range(NHTG):
                    wdt0 = wdpool.tile([128, NH2, 256], F16, name="wdt0",
                                       tag="wd", bufs=4)
                    nc.sync.dma_start(out=wdt0, in_=wd[seg][htg, :, :NH2, :])
                    wdt1 = wdpool.tile([128, NH2, 256], F16, name="wdt1",
                                       tag="wd", bufs=4)
                    nc.sync.dma_start(out=wdt1, in_=wd[seg][htg, :, NH2:, :])
                    py0 = psum.tile([128, L], F32, name="py0", tag="py",
                                    bufs=4, space="PSUM")
                    py1 = psum.tile([128, L], F32, name="py1", tag="py",
                                    bufs=4, space="PSUM")
                    for it in range(NIT):
                        wdt = wdt0 if it < NH2 else wdt1
                        i2 = it % NH2
                        nc.tensor.matmul(py0, lhsT=wdt[:, i2, 0:128],
                                         rhs=h_all[:, it, :],
                                         start=(it == 0), stop=(it == NIT - 1))
                        nc.tensor.matmul(py1, lhsT=wdt[:, i2, 128:256],
                                         rhs=h_all[:, it, :],
                                         start=(it == 0), stop=(it == NIT - 1))
                    ysb = opool.tile([128, 2, L], F32, name="ysb", tag="ysb")
                    nc.vector.tensor_copy(out=ysb[:, 0, :], in_=py0)
                    nc.vector.tensor_copy(out=ysb[:, 1, :], in_=py1)
                    nc.sync.dma_start(
                        out=yt[htg * 256:(htg + 1) * 256,
                               off:off + L].rearrange(
                            "(s p) l -> p s l", p=128),
                        in_=ysb)

            # shared-phase prologue (it=0, chunk=0) emitted in this scope
            # so it overlaps the pool-scope transition below
            pg = psum.tile([128, 512], F32, name="ppg", tag="pg",
                           space="PSUM")
            pu = psum.tile([128, 512], F32, name="ppu", tag="pu",
                           space="PSUM")
            for kt in range(KT):
                nc.tensor.matmul(pg, lhsT=bsg[:, kt, :], rhs=bxc0[:, kt, :],
                                 start=(kt == 0), stop=(kt == KT - 1))
            for kt in range(KT):
                nc.tensor.matmul(pu, lhsT=bsu[:, kt, :], rhs=bxc0[:, kt, :],
                                 start=(kt == 0), stop=(kt == KT - 1))
            gsb = tmppool.tile([128, 512], F16, name="pgsb", tag="gsb")
            nc.scalar.activation(out=gsb, in_=pg, func=SILU)
            nc.vector.tensor_mul(out=bh0, in0=gsb, in1=pu)

        # ---------------- shared experts (TP slice) ----------------
        with (
            tc.tile_pool(name="psum2", bufs=2, space="PSUM") as psum,
            tc.tile_pool(name="xallp", bufs=3) as xapool,
            tc.tile_pool(name="shw", bufs=3) as shwpool,
            tc.tile_pool(name="hshp", bufs=1) as hshpool,
            tc.tile_pool(name="sdp", bufs=2) as sdpool,
            tc.tile_pool(name="tmp2", bufs=3) as tmppool,
            tc.tile_pool(name="osb2", bufs=4) as opool,
        ):
            xc = [bxc0] + [
                xapool.tile([128, KT, 512], F16, name=f"xct{c}", tag="xc")
                for c in range(1, NCH)]
            hsh = hshpool.tile([128, SIT, NCH, 512], F16, name="hsh",
                               tag="hsh")
            # B1: shared swiglu intermediate for all tokens
            # (it=0, c=0 was computed in the routed scope via bridge tiles)
            for it in range(SIT):
                if it == 0:
                    sgt, sut = bsg, bsu
                    for c in range(1, NCH):
                        nc.sync.dma_start(out=xc[c], in_=xall[c])
                    crange = range(1, NCH)
                else:
                    sgt = shwpool.tile([128, KT, 128], F16, name="sgt",
                                       tag="sg")
                    nc.sync.dma_start(out=sgt, in_=sg[it])
                    sut = shwpool.tile([128, KT, 128], F16, name="sut",
                                       tag="su")
                    nc.sync.dma_start(out=sut, in_=su[it])
                    crange = range(NCH)
                for c in crange:
                    pg = psum.tile([128, 512], F32, name="pg2", tag="pg",
                                   space="PSUM")
                    pu = psum.tile([128, 512], F32, name="pu2", tag="pu",
                                   space="PSUM")
                    for kt in range(KT):
                        nc.tensor.matmul(pg, lhsT=sgt[:, kt, :],
                                         rhs=xc[c][:, kt, :],
                                         start=(kt == 0), stop=(kt == KT - 1))
                    for kt in range(KT):
                        nc.tensor.matmul(pu, lhsT=sut[:, kt, :],
                                         rhs=xc[c][:, kt, :],
                                         start=(kt == 0), stop=(kt == KT - 1))
                    gsb = tmppool.tile([128, 512], F16, name="gsb2",
                                       tag="gsb")
                    nc.scalar.activation(out=gsb, in_=pg, func=SILU)
                    nc.vector.tensor_mul(out=hsh[:, it, c, :], in0=gsb,
                                         in1=pu)
            # B2: down-projection of the shared slice
            for htg in range(NHTG):
                sdt = sdpool.tile([128, SIT, 256], F16, name="sdt", tag="sd")
                nc.sync.dma_start(out=sdt, in_=sd[htg])
                for c in range(NCH):
                    py0 = psum.tile([128, 512], F32, name="spy0", tag="py",
                                    bufs=4, space="PSUM")
                    py1 = psum.tile([128, 512], F32, name="spy1", tag="py",
                                    bufs=4, space="PSUM")
                    for it in range(SIT):
                        rhs = bh0 if (it == 0 and c == 0) \
                            else hsh[:, it, c, :]
                        nc.tensor.matmul(py0, lhsT=sdt[:, it, 0:128],
                                         rhs=rhs,
                                         start=(it == 0), stop=(it == SIT - 1))
                        nc.tensor.matmul(py1, lhsT=sdt[:, it, 128:256],
                                         rhs=rhs,
                                         start=(it == 0), stop=(it == SIT - 1))
                    ysb = opool.tile([128, 2, 512], F32, name="ysb2",
                                     tag="ysb")
                    nc.vector.tensor_copy(out=ysb[:, 0, :], in_=py0)
                    nc.vector.tensor_copy(out=ysb[:, 1, :], in_=py1)
                    nc.sync.dma_start(
                        out=ysh[htg * 256:(htg + 1) * 256,
                                c * 512:(c + 1) * 512].rearrange(
                            "(s p) l -> p s l", p=128),
                        in_=ysb)
    nc.compile()
    return nc


def _ensure_ntff_hook():
    """This image's `antenv` package is a stub without `axon_hooks`, which
    makes run_bass_kernel_spmd(trace=True) crash on import even though the
    axon .so supports NTFF capture.  Provide the module and register the
    ctypes-based hook from trn_agent_boot (no-op if anything is missing)."""
    try:
        import antenv.axon_hooks  # noqa: F401
        return
    except ImportError:
        pass
    try:
        import types

        import antenv
        from trn_agent_boot.trn_boot import _ntff_profile_via_ctypes

        mod = types.ModuleType("antenv.axon_hooks")
        mod._hook = None

        def set_axon_ntff_profile_hook(h):
            mod._hook = h

        def get_axon_ntff_profile_hook():
            return mod._hook

        mod.set_axon_ntff_profile_hook = set_axon_ntff_profile_hook
        mod.get_axon_ntff_profile_hook = get_axon_ntff_profile_hook
        sys.modules["antenv.axon_hooks"] = mod
        antenv.axon_hooks = mod
        mod._hook = _ntff_profile_via_ctypes("/opt/axon/libaxon_pjrt.so")
    except Exception:
        pass


LAST_EXEC_NS = None
LAST_RESULTS = None


def kernel(x, w_router, wg, wu, wd, sg, su, sd):
    global LAST_EXEC_NS, LAST_RESULTS
    from concourse import bass_utils

    x = np.asarray(x, dtype=np.float32)
    xt = np.ascontiguousarray(x.reshape(T, H))

    # 1) routing on host (CPU jax, bit-exact with the oracle)
    topi, topw = _route_cpu(xt, np.asarray(w_router, dtype=np.float32))
    counts = np.bincount(topi.reshape(-1), minlength=E)
    LA, LB, asg = _plan_segments(counts)

    # 2) build 8 A-pieces (<= LA tokens) and 8 B-pieces (<= LB tokens),
    #    each single-expert, per the slot assignment
    empty = (0, np.zeros(0, np.int64), np.zeros(0, np.float32))
    piecesA, piecesB = [], []
    for e, a, b in asg:
        ts = np.where((topi == e).any(axis=1))[0]
        w = np.where(topi[ts, 0] == e, topw[ts, 0], topw[ts, 1]).astype(
            np.float32)
        pos = 0
        for _ in range(a):
            n = min(LA, len(ts) - pos)
            if n > 0:
                piecesA.append((e, ts[pos:pos + n], w[pos:pos + n]))
                pos += n
            else:
                piecesA.append(empty)
        for _ in range(b):
            n = min(LB, len(ts) - pos)
            if n > 0:
                piecesB.append((e, ts[pos:pos + n], w[pos:pos + n]))
                pos += n
            else:
                piecesB.append(empty)
        assert pos == len(ts), (e, pos, len(ts))
    assert len(piecesA) <= NCORES and len(piecesB) <= NCORES
    piecesA += [empty] * (NCORES - len(piecesA))
    piecesB += [empty] * (NCORES - len(piecesB))

    # 3) per-core device inputs
    xt16T = np.ascontiguousarray(xt.T).astype(F16NP)        # [H, T]
    wcache = {}

    def expert_pack(e):
        if e not in wcache:
            wcache[e] = (_pack_gu(np.asarray(wg[e])),
                         _pack_gu(np.asarray(wu[e])),
                         _pack_d(np.asarray(wd[e])))
        return wcache[e]

    xall_dev = np.ascontiguousarray(
        xt16T.reshape(KT, 128, NCH, 512).transpose(2, 1, 0, 3))

    SG = np.asarray(sg, dtype=np.float32).reshape(ISH, H)
    SU = np.asarray(su, dtype=np.float32).reshape(ISH, H)
    SD = np.ascontiguousarray(
        np.asarray(sd, dtype=np.float32).transpose(1, 0, 2).reshape(H, ISH))

    in_maps = []
    for k in range(NCORES):
        m = {"xall": xall_dev,
             "sg": _pack_sh_gu(SG[k * ISHC:(k + 1) * ISHC]),
             "su": _pack_sh_gu(SU[k * ISHC:(k + 1) * ISHC]),
             "sd": _pack_sh_d(SD[:, k * ISHC:(k + 1) * ISHC])}
        for sn, L, pieces in (("a", LA, piecesA), ("b", LB, piecesB)):
            e, ts, _ = pieces[k]
            g, u, dn = expert_pack(e)
            m[f"x{sn}"] = _pack_x_slot(xt16T, ts, L)
            m[f"wg{sn}"] = g
            m[f"wu{sn}"] = u
            m[f"wd{sn}"] = dn
        in_maps.append(m)

    # 4) compile + run on the 8 cores
    nc = _build_nc(LA, LB)
    trace = os.environ.get("MOE_TRACE", "0") == "1"
    _ensure_ntff_hook()
    res = bass_utils.run_bass_kernel_spmd(
        nc, in_maps, core_ids=list(range(NCORES)), trace=trace)
    LAST_EXEC_NS = res.exec_time_ns
    LAST_RESULTS = res

    # 5) unshard: weighted scatter-add of routed pieces + sum of shared parts
    outT = np.zeros((H, T), np.float32)
    for k in range(NCORES):
        outT += res.results[k]["ysh"]
    for pieces, off in ((piecesA, 0), (piecesB, LA)):
        for k, (e, ts, w) in enumerate(pieces):
            if len(ts) == 0:
                continue
            yseg = res.results[k]["yt"][:, off:off + len(ts)]   # [H, n]
            outT[:, ts] += yseg * w[None, :]

    return np.ascontiguousarray(outT.T).reshape(B, S, H).astype(np.float32)


# revision 34
# speedup vs baseline: 1.0305x; 1.0115x over previous
"""MoE layer (E=8, top-2, H=2048, I=5120, 2 shared experts) on 8 Trainium2 cores.

Sharding strategy (expert-parallel, per the hint):
  - Router runs on host (it is the *sharding function*: expert-parallel
    dispatch of full inputs requires the routing decision at shard time).
    It is computed with CPU jax using exactly the reference ops so the
    top-k selection (heavily tie-dominated: ~36% of sigmoid scores
    saturate to exactly 1.0) matches the oracle bit-for-bit.
  - The 2*T (token, expert) pairs are split into 16 single-expert pieces
    of <= L tokens; each of the 8 cores gets two pieces (segments A/B),
    each with its own expert-weight inputs.  This load-balances the very
    skewed expert distribution under an SPMD (single-program) kernel.
  - Shared experts are tensor-parallel: the flattened [NSH*I] = 10240
    intermediate dim is sliced 1280 per core; partial outputs summed on
    host during unshard.
  - On-device math: fp16 inputs/weights, fp32 PSUM accumulation
    (measured ~5e-4 max rel err vs the fp32 oracle on host).
  - All device inputs are host-repacked into partition-major tile
    layouts so every DMA is a large per-partition-contiguous transfer.
"""

import os
import subprocess
import sys
import tempfile

import numpy as np

# Problem constants (hardcoded per contract - kernel.py is self-contained).
B, S, H, I, E, TOPK, NSH = 2, 1024, 2048, 5120, 8, 2, 2
T = B * S                      # 2048 tokens
ISH = NSH * I                  # 10240 flattened shared intermediate dim
NCORES = 8
ISHC = ISH // NCORES           # 1280 shared intermediate per core
KT = H // 128                  # 16 contraction tiles over H
NITG = I // 512                # 10 groups of 4 i-tiles
NIT = I // 128                 # 40 i-tiles
NHT = H // 128                 # 16 output h-tiles
NHTG = H // 256                # 8 pairs of h-tiles
SIT = ISHC // 128              # 10 shared i-tiles per core
NCH = T // 512                 # 4 chunks of 512 tokens (shared phase)

F16NP = np.float16

_ROUTE_SRC = """
import jax
jax.config.update("jax_platforms", "cpu")
import jax.numpy as jnp, numpy as np, sys
d = np.load(sys.argv[1])
xt = jnp.asarray(d["xt"]); wr = jnp.asarray(d["wr"])
logits = jnp.einsum("th,eh->te", xt, wr)
scores = jax.nn.sigmoid(logits)
topw, topi = jax.lax.top_k(scores, 2)
topw = topw / jnp.sum(topw, axis=-1, keepdims=True)
np.savez(sys.argv[2], topi=np.asarray(topi), topw=np.asarray(topw))
"""


def _route_cpu(xt, wr):
    """Top-2 routing with CPU jax in a subprocess (the main process's jax
    backend is axon; the oracle's routing is CPU-jax and tie-break
    sensitive, so it must be reproduced bit-exactly on the same backend)."""
    with tempfile.TemporaryDirectory() as td:
        inp = os.path.join(td, "in.npz")
        outp = os.path.join(td, "out.npz")
        src = os.path.join(td, "route.py")
        np.savez(inp, xt=xt, wr=wr)
        with open(src, "w") as f:
            f.write(_ROUTE_SRC)
        env = dict(os.environ)
        env.pop("JAX_PLATFORMS", None)
        subprocess.run([sys.executable, src, inp, outp], check=True, env=env)
        d = np.load(outp)
        return d["topi"], d["topw"]


def _try_assign(counts, LA, LB):
    """Assign each expert (a_e, b_e) slot counts with a_e*LA + b_e*LB >=
    N_e, sum(a) <= 8, sum(b) <= 8.  DFS over experts, largest first."""
    order = sorted(range(E), key=lambda e: -counts[e])

    def rec(idx, remA, remB, acc):
        if idx == len(order):
            return acc
        e = order[idx]
        n = int(counts[e])
        for a in range(min(remA, -(-n // LA)) + 1):
            rem = n - a * LA
            b = max(0, -(-rem // LB))
            if b > remB:
                continue
            r = rec(idx + 1, remA - a, remB - b, acc + [(e, a, b)])
            if r is not None:
                return r
        return None

    return rec(0, NCORES, NCORES, [])


def _plan_segments(counts):
    """Smallest per-core capacity (LA, LB) with a feasible slot assignment.
    LA/LB are compile-time segment lengths (<=512, multiples of 64)."""
    for tot in range(128, 2 * 512 + 1, 64):
        for LB in range(min(tot // 2 // 64 * 64, 512), 63, -64):
            LA = tot - LB
            if LA > 512:
                continue
            asg = _try_assign(counts, LA, LB)
            if asg is not None:
                return LA, LB, asg
    raise ValueError(f"cannot pack counts {counts}")


# ------------- host-side packing into partition-major DMA layouts -----------

def _pack_gu(w):            # w [I, H] f32 -> [NITG, 128, KT, 512] f16
    wT = np.ascontiguousarray(w.T).astype(F16NP)            # [H, I]
    return np.ascontiguousarray(
        wT.reshape(KT, 128, NITG, 512).transpose(2, 1, 0, 3))


def _pack_d(w):             # w [H, I] f32 -> [NHTG, 128, NIT, 256] f16
    wT = np.ascontiguousarray(w.T).astype(F16NP)            # [I, H]
    return np.ascontiguousarray(
        wT.reshape(NIT, 128, NHTG, 256).transpose(2, 1, 0, 3))


def _pack_sh_gu(sgc):       # sgc [ISHC, H] f32 -> [SIT, 128, KT, 128] f16
    sT = np.ascontiguousarray(sgc.T).astype(F16NP)          # [H, ISHC]
    return np.ascontiguousarray(
        sT.reshape(KT, 128, SIT, 128).transpose(2, 1, 0, 3))


def _pack_sh_d(sdc):        # sdc [H, ISHC] f32 -> [NHTG, 128, SIT, 256] f16
    sT = np.ascontiguousarray(sdc.T).astype(F16NP)          # [ISHC, H]
    return np.ascontiguousarray(
        sT.reshape(SIT, 128, NHTG, 256).transpose(2, 1, 0, 3))


def _pack_x_slot(xt16T, tokens, L):   # -> [128, KT, L] f16
    xs = np.zeros((H, L), F16NP)
    if len(tokens):
        xs[:, : len(tokens)] = xt16T[:, tokens]
    return np.ascontiguousarray(xs.reshape(KT, 128, L).transpose(1, 0, 2))


# --------------------------- device program ---------------------------------

def _build_nc(LA, LB):
    import concourse.mybir as mybir
    import concourse.tile as tile
    from concourse import bacc

    F16 = mybir.dt.float16
    F32 = mybir.dt.float32
    SILU = mybir.ActivationFunctionType.Silu

    nc = bacc.Bacc("TRN2", debug=False, num_devices=NCORES)

    def din(name, shape):
        return nc.dram_tensor(name, shape, F16, kind="ExternalInput").ap()

    xa = din("xa", [128, KT, LA])
    xb = din("xb", [128, KT, LB])
    wg = [din(f"wg{s}", [NITG, 128, KT, 512]) for s in "ab"]
    wu = [din(f"wu{s}", [NITG, 128, KT, 512]) for s in "ab"]
    wd = [din(f"wd{s}", [NHTG, 128, NIT, 256]) for s in "ab"]
    xall = din("xall", [NCH, 128, KT, 512])
    sg = din("sg", [SIT, 128, KT, 128])
    su = din("su", [SIT, 128, KT, 128])
    sd = din("sd", [NHTG, 128, SIT, 256])

    yt = nc.dram_tensor("yt", [H, LA + LB], F32, kind="ExternalOutput").ap()
    ysh = nc.dram_tensor("ysh", [H, T], F32, kind="ExternalOutput").ap()

    with tile.TileContext(nc) as tc, \
            tc.tile_pool(name="bridge", bufs=1) as bridge:
        # bridge tiles span the routed->shared pool-scope boundary so the
        # shared phase's first iteration has its inputs prefetched and its
        # compute emitted inside the routed scope (no transition stall)
        bxc0 = bridge.tile([128, KT, 512], F16, name="bxc0", tag="bxc")
        bsg = bridge.tile([128, KT, 128], F16, name="bsg", tag="bsg")
        bsu = bridge.tile([128, KT, 128], F16, name="bsu", tag="bsu")
        bh0 = bridge.tile([128, 512], F16, name="bh0", tag="bh0")
        # ---------------- routed experts: two segments ----------------
        with (
            tc.tile_pool(name="psum", bufs=2, space="PSUM") as psum,
            tc.tile_pool(name="xslot", bufs=2) as xpool,
            tc.tile_pool(name="wgu", bufs=2) as wpool,
            tc.tile_pool(name="wdp", bufs=2) as wdpool,
            tc.tile_pool(name="hp", bufs=2) as hpool,
            tc.tile_pool(name="tmp", bufs=3) as tmppool,
            tc.tile_pool(name="osb", bufs=2) as opool,
        ):
            for seg, (xs_d, L, off) in enumerate(
                    [(xa, LA, 0), (xb, LB, LA)]):
                xs = xpool.tile([128, KT, L], F16, name="xs", tag="xs")
                if seg > 0:
                    nc.sync.dma_start(out=xs, in_=xs_d)
                h_all = hpool.tile([128, NIT, L], F16, name="h_all", tag="h")
                # stage 1: h = silu(x@wgT) * (x@wuT), laid out [I, L]
                for itg in range(NITG):
                    wgt = wpool.tile([128, KT, 512], F16, name="wgt", tag="wg")
                    wut = wpool.tile([128, KT, 512], F16, name="wut", tag="wu")
                    if seg == 0 and itg == 0:
                        # interleave first loads in 4-ktile chunks so the
                        # first matmuls' inputs arrive as early as possible
                        for q in range(4):
                            ksl = slice(q * 4, (q + 1) * 4)
                            nc.sync.dma_start(out=xs[:, ksl, :],
                                              in_=xs_d[:, ksl, :])
                            nc.sync.dma_start(out=wgt[:, ksl, :],
                                              in_=wg[seg][itg, :, ksl, :])
                            nc.sync.dma_start(out=wut[:, ksl, :],
                                              in_=wu[seg][itg, :, ksl, :])
                    else:
                        nc.sync.dma_start(out=wgt, in_=wg[seg][itg])
                        nc.sync.dma_start(out=wut, in_=wu[seg][itg])
                    if seg == 1 and itg == 0:
                        # prefetch the shared-phase bridge inputs; emitted
                        # here (not at the head) so they don't compete with
                        # the critical first weight streams
                        nc.sync.dma_start(out=bxc0, in_=xall[0])
                        nc.sync.dma_start(out=bsg, in_=sg[0])
                        nc.sync.dma_start(out=bsu, in_=su[0])
                    for it4 in range(4):
                        it = itg * 4 + it4
                        pg = psum.tile([128, L], F32, name="pg", tag="pg",
                                       space="PSUM")
                        pu = psum.tile([128, L], F32, name="pu", tag="pu",
                                       space="PSUM")
                        csl = slice(it4 * 128, (it4 + 1) * 128)
                        for kt in range(KT):
                            nc.tensor.matmul(pg, lhsT=wgt[:, kt, csl],
                                             rhs=xs[:, kt, :],
                                             start=(kt == 0),
                                             stop=(kt == KT - 1))
                        for kt in range(KT):
                            nc.tensor.matmul(pu, lhsT=wut[:, kt, csl],
                                             rhs=xs[:, kt, :],
                                             start=(kt == 0),
                                             stop=(kt == KT - 1))
                        gsb = tmppool.tile([128, L], F16, name="gsb",
                                           tag="gsb")
                        nc.scalar.activation(out=gsb, in_=pg, func=SILU)
                        nc.vector.tensor_mul(out=h_all[:, it, :], in0=gsb,
                                             in1=pu)
                # stage 2: y = h @ wdT, output [H, L].  wd streamed in
                # half-tiles (finer prefetch rotation hides DMA latency)
                NH2 = NIT // 2
                for htg in range(NHTG):
                    wdt0 = wdpool.tile([128, NH2, 256], F16, name="wdt0",
                                       tag="wd", bufs=4)
                    nc.sync.dma_start(out=wdt0, in_=wd[seg][htg, :, :NH2, :])
                    wdt1 = wdpool.tile([128, NH2, 256], F16, name="wdt1",
                                       tag="wd", bufs=4)
                    nc.sync.dma_start(out=wdt1, in_=wd[seg][htg, :, NH2:, :])
                    py0 = psum.tile([128, L], F32, name="py0", tag="py",
                                    bufs=4, space="PSUM")
                    py1 = psum.tile([128, L], F32, name="py1", tag="py",
                                    bufs=4, space="PSUM")
                    for it in range(NIT):
                        wdt = wdt0 if it < NH2 else wdt1
                        i2 = it % NH2
                        nc.tensor.matmul(py0, lhsT=wdt[:, i2, 0:128],
                                         rhs=h_all[:, it, :],
                                         start=(it == 0), stop=(it == NIT - 1))
                        nc.tensor.matmul(py1, lhsT=wdt[:, i2, 128:256],
                                         rhs=h_all[:, it, :],
                                         start=(it == 0), stop=(it == NIT - 1))
                    ysb = opool.tile([128, 2, L], F32, name="ysb", tag="ysb")
                    nc.vector.tensor_copy(out=ysb[:, 0, :], in_=py0)
                    nc.vector.tensor_copy(out=ysb[:, 1, :], in_=py1)
                    nc.sync.dma_start(
                        out=yt[htg * 256:(htg + 1) * 256,
                               off:off + L].rearrange(
                            "(s p) l -> p s l", p=128),
                        in_=ysb)

            # shared-phase prologue (it=0, chunk=0) emitted in this scope
            # so it overlaps the pool-scope transition below
            pg = psum.tile([128, 512], F32, name="ppg", tag="pg",
                           space="PSUM")
            pu = psum.tile([128, 512], F32, name="ppu", tag="pu",
                           space="PSUM")
            for kt in range(KT):
                nc.tensor.matmul(pg, lhsT=bsg[:, kt, :], rhs=bxc0[:, kt, :],
                                 start=(kt == 0), stop=(kt == KT - 1))
            for kt in range(KT):
                nc.tensor.matmul(pu, lhsT=bsu[:, kt, :], rhs=bxc0[:, kt, :],
                                 start=(kt == 0), stop=(kt == KT - 1))
            gsb = tmppool.tile([128, 512], F16, name="pgsb", tag="gsb")
            nc.scalar.activation(out=gsb, in_=pg, func=SILU)
            nc.vector.tensor_mul(out=bh0, in0=gsb, in1=pu)

        # ---------------- shared experts (TP slice) ----------------
        with (
            tc.tile_pool(name="psum2", bufs=2, space="PSUM") as psum,
            tc.tile_pool(name="xallp", bufs=3) as xapool,
            tc.tile_pool(name="shw", bufs=3) as shwpool,
            tc.tile_pool(name="hshp", bufs=1) as hshpool,
            tc.tile_pool(name="sdp", bufs=2) as sdpool,
            tc.tile_pool(name="tmp2", bufs=3) as tmppool,
            tc.tile_pool(name="osb2", bufs=4) as opool,
        ):
            xc = [bxc0] + [
                xapool.tile([128, KT, 512], F16, name=f"xct{c}", tag="xc")
                for c in range(1, NCH)]
            hsh = hshpool.tile([128, SIT, NCH, 512], F16, name="hsh",
                               tag="hsh")
            # B1: shared swiglu intermediate for all tokens
            # (it=0, c=0 was computed in the routed scope via bridge tiles)
            for it in range(SIT):
                if it == 0:
                    sgt, sut = bsg, bsu
                    for c in range(1, NCH):
                        nc.sync.dma_start(out=xc[c], in_=xall[c])
                    crange = range(1, NCH)
                else:
                    sgt = shwpool.tile([128, KT, 128], F16, name="sgt",
                                       tag="sg")
                    nc.sync.dma_start(out=sgt, in_=sg[it])
                    sut = shwpool.tile([128, KT, 128], F16, name="sut",
                                       tag="su")
                    nc.sync.dma_start(out=sut, in_=su[it])
                    crange = range(NCH)
                for c in crange:
                    pg = psum.tile([128, 512], F32, name="pg2", tag="pg",
                                   space="PSUM")
                    pu = psum.tile([128, 512], F32, name="pu2", tag="pu",
                                   space="PSUM")
                    for kt in range(KT):
                        nc.tensor.matmul(pg, lhsT=sgt[:, kt, :],
                                         rhs=xc[c][:, kt, :],
                                         start=(kt == 0), stop=(kt == KT - 1))
                    for kt in range(KT):
                        nc.tensor.matmul(pu, lhsT=sut[:, kt, :],
                                         rhs=xc[c][:, kt, :],
                                         start=(kt == 0), stop=(kt == KT - 1))
                    gsb = tmppool.tile([128, 512], F16, name="gsb2",
                                       tag="gsb")
                    nc.scalar.activation(out=gsb, in_=pg, func=SILU)
                    nc.vector.tensor_mul(out=hsh[:, it, c, :], in0=gsb,
                                         in1=pu)
            # B2: down-projection of the shared slice
            for htg in range(NHTG):
                sdt = sdpool.tile([128, SIT, 256], F16, name="sdt", tag="sd")
                nc.sync.dma_start(out=sdt, in_=sd[htg])
                for c in range(NCH):
                    py0 = psum.tile([128, 512], F32, name="spy0", tag="py",
                                    bufs=4, space="PSUM")
                    py1 = psum.tile([128, 512], F32, name="spy1", tag="py",
                                    bufs=4, space="PSUM")
                    for it in range(SIT):
                        rhs = bh0 if (it == 0 and c == 0) \
                            else hsh[:, it, c, :]
                        nc.tensor.matmul(py0, lhsT=sdt[:, it, 0:128],
                                         rhs=rhs,
                                         start=(it == 0), stop=(it == SIT - 1))
                        nc.tensor.matmul(py1, lhsT=sdt[:, it, 128:256],
                                         rhs=rhs,
                                         start=(it == 0), stop=(it == SIT - 1))
                    ysb = opool.tile([128, 2, 512], F32, name="ysb2",
                                     tag="ysb")
                    nc.vector.tensor_copy(out=ysb[:, 0, :], in_=py0)
                    nc.vector.tensor_copy(out=ysb[:, 1, :], in_=py1)
                    nc.sync.dma_start(
                        out=ysh[htg * 256:(htg + 1) * 256,
                                c * 512:(c + 1) * 512].rearrange(
                            "(s p) l -> p s l", p=128),
                        in_=ysb)
    nc.compile()
    return nc


def _ensure_ntff_hook():
    """This image's `antenv` package is a stub without `axon_hooks`, which
    makes run_bass_kernel_spmd(trace=True) crash on import even though the
    axon .so supports NTFF capture.  Provide the module and register the
    ctypes-based hook from trn_agent_boot (no-op if anything is missing)."""
    try:
        import antenv.axon_hooks  # noqa: F401
        return
    except ImportError:
        pass
    try:
        import types

        import antenv
        from trn_agent_boot.trn_boot import _ntff_profile_via_ctypes

        mod = types.ModuleType("antenv.axon_hooks")
        mod._hook = None

        def set_axon_ntff_profile_hook(h):
            mod._hook = h

        def get_axon_ntff_profile_hook():
            return mod._hook

        mod.set_axon_ntff_profile_hook = set_axon_ntff_profile_hook
        mod.get_axon_ntff_profile_hook = get_axon_ntff_profile_hook
        sys.modules["antenv.axon_hooks"] = mod
        antenv.axon_hooks = mod
        mod._hook = _ntff_profile_via_ctypes("/opt/axon/libaxon_pjrt.so")
    except Exception:
        pass


LAST_EXEC_NS = None
LAST_RESULTS = None


def kernel(x, w_router, wg, wu, wd, sg, su, sd):
    global LAST_EXEC_NS, LAST_RESULTS
    from concourse import bass_utils

    x = np.asarray(x, dtype=np.float32)
    xt = np.ascontiguousarray(x.reshape(T, H))

    # 1) routing on host (CPU jax, bit-exact with the oracle)
    topi, topw = _route_cpu(xt, np.asarray(w_router, dtype=np.float32))
    counts = np.bincount(topi.reshape(-1), minlength=E)
    LA, LB, asg = _plan_segments(counts)

    # 2) build 8 A-pieces (<= LA tokens) and 8 B-pieces (<= LB tokens),
    #    each single-expert, per the slot assignment
    empty = (0, np.zeros(0, np.int64), np.zeros(0, np.float32))
    piecesA, piecesB = [], []
    for e, a, b in asg:
        ts = np.where((topi == e).any(axis=1))[0]
        w = np.where(topi[ts, 0] == e, topw[ts, 0], topw[ts, 1]).astype(
            np.float32)
        pos = 0
        for _ in range(a):
            n = min(LA, len(ts) - pos)
            if n > 0:
                piecesA.append((e, ts[pos:pos + n], w[pos:pos + n]))
                pos += n
            else:
                piecesA.append(empty)
        for _ in range(b):
            n = min(LB, len(ts) - pos)
            if n > 0:
                piecesB.append((e, ts[pos:pos + n], w[pos:pos + n]))
                pos += n
            else:
                piecesB.append(empty)
        assert pos == len(ts), (e, pos, len(ts))
    assert len(piecesA) <= NCORES and len(piecesB) <= NCORES
    piecesA += [empty] * (NCORES - len(piecesA))
    piecesB += [empty] * (NCORES - len(piecesB))

    # 3) per-core device inputs
    xt16T = np.ascontiguousarray(xt.T).astype(F16NP)        # [H, T]
    wcache = {}

    def expert_pack(e):
        if e not in wcache:
            wcache[e] = (_pack_gu(np.asarray(wg[e])),
                         _pack_gu(np.asarray(wu[e])),
                         _pack_d(np.asarray(wd[e])))
        return wcache[e]

    xall_dev = np.ascontiguousarray(
        xt16T.reshape(KT, 128, NCH, 512).transpose(2, 1, 0, 3))

    SG = np.asarray(sg, dtype=np.float32).reshape(ISH, H)
    SU = np.asarray(su, dtype=np.float32).reshape(ISH, H)
    SD = np.ascontiguousarray(
        np.asarray(sd, dtype=np.float32).transpose(1, 0, 2).reshape(H, ISH))

    in_maps = []
    for k in range(NCORES):
        m = {"xall": xall_dev,
             "sg": _pack_sh_gu(SG[k * ISHC:(k + 1) * ISHC]),
             "su": _pack_sh_gu(SU[k * ISHC:(k + 1) * ISHC]),
             "sd": _pack_sh_d(SD[:, k * ISHC:(k + 1) * ISHC])}
        for sn, L, pieces in (("a", LA, piecesA), ("b", LB, piecesB)):
            e, ts, _ = pieces[k]
            g, u, dn = expert_pack(e)
            m[f"x{sn}"] = _pack_x_slot(xt16T, ts, L)
            m[f"wg{sn}"] = g
            m[f"wu{sn}"] = u
            m[f"wd{sn}"] = dn
        in_maps.append(m)

    # 4) compile + run on the 8 cores
    nc = _build_nc(LA, LB)
    trace = os.environ.get("MOE_TRACE", "0") == "1"
    _ensure_ntff_hook()
    res = bass_utils.run_bass_kernel_spmd(
        nc, in_maps, core_ids=list(range(NCORES)), trace=trace)
    LAST_EXEC_NS = res.exec_time_ns
    LAST_RESULTS = res

    # 5) unshard: weighted scatter-add of routed pieces + sum of shared parts
    outT = np.zeros((H, T), np.float32)
    for k in range(NCORES):
        outT += res.results[k]["ysh"]
    for pieces, off in ((piecesA, 0), (piecesB, LA)):
        for k, (e, ts, w) in enumerate(pieces):
            if len(ts) == 0:
                continue
            yseg = res.results[k]["yt"][:, off:off + len(ts)]   # [H, n]
            outT[:, ts] += yseg * w[None, :]

    return np.ascontiguousarray(outT.T).reshape(B, S, H).astype(np.float32)
